# revision 29
# baseline (speedup 1.0000x reference)
"""Trainium2 Bass kernel for nn_Attention (B=2, N=2048, H=16, hd=64, D=1024).

Strategy (8 NeuronCores):
  core c -> batch b=c//4, 4 heads (4r..4r+3, r=c%4). Each core computes
  Q,K,V for its 4 heads over its batch (K over the first KP=KT*128 key
  rows; KT specialized to runtime vaild_num; masked tail keys get bias
  -1e9 before exp -> P=0), attention in transposed layout (S^T[k,q]),
  and SENDER-SIDE softmax normalization: the denominator rides along the
  PV matmul as diag-ones columns landing on PSUM rows 64..67, one
  reciprocal per query chunk, broadcast to 64 partitions via a tiny
  matmul, multiplied into U. Normalized U blocks are exchanged with an
  in-batch AllToAll (replica groups [[0..3],[4..7]]); the receiver only
  loads head-pair-stacked U tiles and runs the output projection with
  full 128-deep contraction.
  Per-sample valid-length semantics:
    - keys k >= v: bias -1e9 before exp -> P=0
    - queries q >= v: denominator += 1e30 (host-precomputed row) so the
      normalized U rows ~ 0; the reference's uniform-softmax output
      (mean(V) @ W_proj) is added as a host-precomputed rank-1 fixup row
      in the projection matmul.
  A chain of dependency-free dummy matmuls keeps the PE HAM clock
  un-throttled across the collective window.

Compute dtype bf16 (fp32 PSUM accumulation); fp32 in/out.
"""

import numpy as np
import ml_dtypes

import concourse.mybir as mybir
import concourse.tile as tile
from concourse import bacc
from concourse.bass_utils import run_bass_kernel_spmd

F32 = mybir.dt.float32
BF16 = mybir.dt.bfloat16
AF = mybir.ActivationFunctionType
OP = mybir.AluOpType

H, HD, D, N, B, NCORES = 16, 64, 1024, 2048, 2, 8
QC = 512            # query rows per core
NEG = -1e9
BF = ml_dtypes.bfloat16
BSn = 4 * 64 * QC   # per-dest A2A payload (4 head blocks of [64, QC])
NDUMMY = 150        # PE warm-keeping matmuls across the collective


def build_nc(KT, BT0):
    NPAIR = BT0 // 2
    KP = KT * 128
    kchunks = []
    off = 0
    while off < KP:
        w = min(512, KP - off)
        kchunks.append((off, w))
        off += w
    NST = KT
    VW = 64 + 82 * 4   # 4 local head blocks, stride 81, diag col at 82l+64

    nc = bacc.Bacc(None, target_bir_lowering=False)

    xT_d = nc.declare_dram_parameter("xT", [D, N], BF16, isOutput=False)
    wqmy_d = nc.declare_dram_parameter("wqmy", [D, 256], BF16, isOutput=False)
    wkmy_d = nc.declare_dram_parameter("wkmy", [D, 256], BF16, isOutput=False)
    wvmy_d = nc.declare_dram_parameter("wvmy", [D, 256], BF16, isOutput=False)
    bqmy_d = nc.declare_dram_parameter("bqmy", [128, 2], F32, isOutput=False)
    bkmy_d = nc.declare_dram_parameter("bkmy", [128, 2], F32, isOutput=False)
    bvrowmy_d = nc.declare_dram_parameter("bvrowmy", [1, 256], BF16, isOutput=False)
    wproj_d = nc.declare_dram_parameter("wproj", [D, D], BF16, isOutput=False)
    bprow_d = nc.declare_dram_parameter("bprow", [1, D], BF16, isOutput=False)
    fixrow_d = nc.declare_dram_parameter("fixrow", [1, D], BF16, isOutput=False)
    iqrow_d = nc.declare_dram_parameter("iqrow", [1, QC], BF16, isOutput=False)
    qinf_d = nc.declare_dram_parameter("qinf", [4, N], F32, isOutput=False)
    esel4_d = nc.declare_dram_parameter("esel4", [4, 512], BF16, isOutput=False)
    v128_d = nc.declare_dram_parameter("v128", [128, 1], F32, isOutput=False)
    kiota_d = nc.declare_dram_parameter("kiota", [128, KT], F32, isOutput=False)
    out_d = nc.declare_dram_parameter("out", [QC, D], F32, isOutput=True)

    with tile.TileContext(nc) as tc:
        with tc.tile_pool(name="const", bufs=1) as cpool, \
             tc.tile_pool(name="qkv", bufs=1) as qkvpool, \
             tc.tile_pool(name="wpp", bufs=1) as wppool, \
             tc.tile_pool(name="psA", bufs=2, space="PSUM") as psA, \
             tc.tile_pool(name="psBig", bufs=2, space="PSUM") as psBig, \
             tc.tile_pool(name="psPV", bufs=2, space="PSUM") as psPV:

            # ---------------- constants ----------------
            bqmy = cpool.tile([128, 2], F32, tag="bqmy")
            bkmy = cpool.tile([128, 2], F32, tag="bkmy")
            v128 = cpool.tile([128, 1], F32, tag="v128")
            kiota = cpool.tile([128, KT], F32, tag="kiota")
            bvrowmy = cpool.tile([1, 256], BF16, tag="bvrowmy")
            nc.sync.dma_start(out=bkmy[:, :], in_=bkmy_d[:, :])
            nc.sync.dma_start(out=bqmy[:, :], in_=bqmy_d[:, :])
            nc.gpsimd.dma_start(out=v128[:, :], in_=v128_d[:, :])
            nc.gpsimd.dma_start(out=kiota[:, :], in_=kiota_d[:, :])
            nc.gpsimd.dma_start(out=bvrowmy[:, :], in_=bvrowmy_d[:, :])
            ones1 = cpool.tile([1, 128], BF16, tag="ones1")
            nc.vector.memset(ones1[:, :], 1.0)
            kb = cpool.tile([128, KT], F32, tag="kb")
            nc.vector.tensor_scalar(out=kb[:, :], in0=kiota[:, :],
                                    scalar1=v128[:, 0:1], scalar2=NEG,
                                    op0=OP.is_ge, op1=OP.mult)
            qinf = cpool.tile([128, N], F32, tag="qinf")
            nc.gpsimd.dma_start(out=qinf[64:68, :], in_=qinf_d[:, :])
            esel4 = cpool.tile([128, 512], BF16, tag="esel4")
            nc.gpsimd.dma_start(out=esel4[64:68, :], in_=esel4_d[:, :])
            iqrow = cpool.tile([1, QC], BF16, tag="iqrow")
            nc.gpsimd.dma_start(out=iqrow[:, :], in_=iqrow_d[:, :])
            bprow = cpool.tile([1, D], BF16, tag="bprow")
            nc.gpsimd.dma_start(out=bprow[:, :], in_=bprow_d[:, :])
            fixrow = cpool.tile([1, D], BF16, tag="fixrow")
            nc.gpsimd.dma_start(out=fixrow[:, :], in_=fixrow_d[:, :])
            onesq = cpool.tile([1, QC], BF16, tag="onesq")
            nc.vector.memset(onesq[:, :], 1.0)
            warm0 = cpool.tile([1, 512], BF16, tag="warm0")
            nc.vector.memset(warm0[:, :], 0.0)

            wp2 = [wppool.tile([128, D], BF16, tag=f"wp{p}", name=f"wp{p}") for p in range(8)]
            ktil = [qkvpool.tile([128, KP], BF16, tag=f"kt{i}", name=f"kt{i}") for i in range(2)]
            qtil = [qkvpool.tile([128, N], BF16, tag=f"qt{i}", name=f"qt{i}") for i in range(2)]
            vaug = [qkvpool.tile([128, VW], BF16, tag=f"va{s}", name=f"va{s}") for s in range(NST)]

            # A2A payload: per destination rank: U[4 heads][64, QC]; slots for
            # the other batch's cores carry flag-zeroed data (SPMD program).
            with tc.tile_pool(name="dram", bufs=1, space="DRAM") as dpool:
                shard = dpool.tile([8 * BSn], BF16, tag="shard")
                gath = dpool.tile([8 * BSn], BF16, tag="gath")

            with tc.tile_pool(name="xp", bufs=1) as xpool:
                xT = [xpool.tile([128, N], BF16, tag=f"xT{i}", name=f"xT{i}") for i in range(8)]
                wkmy = [xpool.tile([128, 256], BF16, tag=f"wk{i}", name=f"wk{i}") for i in range(8)]
                wvmy = [xpool.tile([128, 256], BF16, tag=f"wv{i}", name=f"wv{i}") for i in range(8)]
                wqmy = [xpool.tile([128, 256], BF16, tag=f"wq{i}", name=f"wq{i}") for i in range(8)]
                # V-phase deps stream first on the sync queue (V doubles as the
                # PE warm-up); K/Q weights on gpsimd
                for i in range(8):
                    nc.sync.dma_start(out=xT[i][:, :], in_=xT_d[128 * i:128 * (i + 1), :])
                    nc.sync.dma_start(out=wvmy[i][:, :], in_=wvmy_d[128 * i:128 * (i + 1), :])
                for i in range(8):
                    nc.gpsimd.dma_start(out=wkmy[i][:, :], in_=wkmy_d[128 * i:128 * (i + 1), :])
                for i in range(8):
                    nc.gpsimd.dma_start(out=wqmy[i][:, :], in_=wqmy_d[128 * i:128 * (i + 1), :])
                for p in range(8):
                    nc.gpsimd.dma_start(out=wp2[p][:, :], in_=wproj_d[128 * p:128 * (p + 1), :])

                # ---- local K^T / Q^T for head pair 0; pair 1 is deferred and
                # interleaved under chunk 0's ACT-bound attention groups ----
                def emit_k(i, coff, cw, ps=None, xks=range(8)):
                    if ps is None:
                        ps = psA.tile([128, 512], F32, tag="psA")
                    for xk in xks:
                        nc.tensor.matmul(ps[:, 0:cw],
                                         wkmy[xk][:, 128 * i:128 * (i + 1)],
                                         xT[xk][:, coff:coff + cw],
                                         start=(xk == 0), stop=(xk == 7))
                    if xks[-1] == 7:
                        nc.scalar.activation(ktil[i][:, coff:coff + cw], ps[:, 0:cw],
                                             AF.Identity, bias=bkmy[:, i:i + 1])
                    return ps

                def emit_q(i, qc4, ps=None, xks=range(8)):
                    if ps is None:
                        ps = psA.tile([128, 512], F32, tag="psA")
                    for xk in xks:
                        nc.tensor.matmul(ps[:, :],
                                         wqmy[xk][:, 128 * i:128 * (i + 1)],
                                         xT[xk][:, 512 * qc4:512 * (qc4 + 1)],
                                         start=(xk == 0), stop=(xk == 7))
                    if xks[-1] == 7:
                        nc.scalar.activation(qtil[i][:, 512 * qc4:512 * (qc4 + 1)], ps[:, :],
                                             AF.Identity, bias=bqmy[:, i:i + 1],
                                             scale=1.0 / 8.0)
                    return ps

                # ---- local V (4 heads, augmented) — emitted first: its
                # matmuls stream with the input DMAs and double as the PE
                # HAM warm-up ----
                for st in range(NST):
                    nc.vector.memset(vaug[st][:, :], 0.0)
                    diag = vaug[st][:, 64:64 + 82 * 4].rearrange("p (h c) -> p h c", c=82)[:, :, 0:1]
                    nc.vector.memset(diag, 1.0)
                    ps = psBig.tile([128, 2 * QC], F32, tag="psBig")
                    for xk in range(8):
                        nc.tensor.matmul(ps[:, 0:256],
                                         xT[xk][:, 128 * st:128 * (st + 1)],
                                         wvmy[xk][:, :],
                                         start=(xk == 0), stop=False)
                    nc.tensor.matmul(ps[:, 0:256], ones1[:, :], bvrowmy[:, :],
                                     start=False, stop=True)
                    dst = vaug[st][:, 0:81 * 4].rearrange("p (h c) -> p h c", c=81)[:, :, 0:64]
                    nc.vector.tensor_copy(out=dst, in_=ps[:, 0:256])

                # K pair 0 + Q pair 0 chunk 0 are all attention needs to
                # start; the rest is deferred into ACT-bound attention slots.
                for (coff, cw) in kchunks:
                    emit_k(0, coff, cw)
                emit_q(0, 0)
                # (deadline, kind, i, args...): deadline = j*4+l index before
                # whose score groups the item must be complete
                deferred = []
                for (coff, cw) in kchunks:
                    deferred.append((2, "k", 1, coff, cw))
                deferred.append((2, "q", 1, 0))
                for qc4 in range(1, 4):
                    deferred.append((4 * qc4, "q", 0, qc4))
                for qc4 in range(1, 4):
                    deferred.append((4 * qc4 + 2, "q", 1, qc4))
                dstate = {"ps": None, "item": None, "half": 0}

                def pop_deferred():
                    if dstate["item"] is None:
                        if not deferred:
                            return
                        dstate["item"] = deferred.pop(0)
                        dstate["half"] = 0
                    it = dstate["item"]
                    xks = range(4) if dstate["half"] == 0 else range(4, 8)
                    if it[1] == "k":
                        dstate["ps"] = emit_k(it[2], it[3], it[4], ps=dstate["ps"], xks=xks)
                    else:
                        dstate["ps"] = emit_q(it[2], it[3], ps=dstate["ps"], xks=xks)
                    if dstate["half"] == 0:
                        dstate["half"] = 1
                    else:
                        dstate["item"] = None
                        dstate["ps"] = None

                def force_due(idx):
                    while True:
                        nxt = dstate["item"] or (deferred[0] if deferred else None)
                        if nxt is not None and nxt[0] <= idx:
                            pop_deferred()
                        else:
                            break

                # ---------------- attention (4 q-chunks x 4 local heads) ----
                with tc.tile_pool(name="attn0", bufs=1) as a0pool, \
                     tc.tile_pool(name="ppool", bufs=2) as ppool:
                    dblk = [a0pool.tile([128, QC], F32, tag=f"db{j}", name=f"db{j}")
                            for j in range(4)]
                    for j in range(4):
                        nc.vector.memset(dblk[j][64:68, :], 0.0)
                    # U stash, parity-alternated by j so chunk j's flush can be
                    # emitted during chunk j+1 without versioning hazards
                    usb = [a0pool.tile([64, QC], BF16, tag=f"usb{i}", name=f"usb{i}")
                           for i in range(8)]

                    def flush(jj, rmk):
                        """Emit rb broadcasts + U normalize + shard stores for
                        chunk jj (PE-light; rmk must be ready by exec time)."""
                        ub = usb[4 * (jj % 2):4 * (jj % 2) + 4]
                        for l in range(4):
                            for half in range(2):
                                rb = psA.tile([64, QC], F32, tag="psA")
                                nc.tensor.matmul(
                                    rb[:, :],
                                    esel4[64:68, 256 * half + 64 * l:256 * half + 64 * l + 64],
                                    rmk[64:68, :], start=True, stop=True)
                                usnd = a0pool.tile([64, QC], BF16, tag="usnd",
                                                   bufs=4, name="usnd")
                                nc.vector.tensor_tensor(out=usnd[:, :], in0=ub[l][:, :],
                                                        in1=rb[:, :], op=OP.mult)
                                blk = BSn * (jj + 4 * half)
                                nc.sync.dma_start(
                                    out=shard[blk + 64 * QC * l:blk + 64 * QC * (l + 1)],
                                    in_=usnd[:, :])

                    pending = None
                    for j in range(4):
                        for l in range(4):
                            force_due(4 * j + l)
                            ftl, hb = l // 2, 64 * (l % 2)
                            ptil = {}
                            for pi in range(NPAIR):
                                ps = psBig.tile([128, 2 * QC], F32, tag="psBig")
                                for s in range(2):
                                    kt = 2 * pi + s
                                    nc.tensor.matmul(ps[:, QC * s:QC * (s + 1)],
                                                     ktil[ftl][hb:hb + 64, 128 * kt:128 * (kt + 1)],
                                                     qtil[ftl][hb:hb + 64, QC * j:QC * (j + 1)],
                                                     start=True, stop=True)
                                pt = ppool.tile([128, 2 * QC], BF16, tag="ptp", bufs=8, name="pt")
                                nc.scalar.activation(pt[:, :], ps[:, :], AF.Exp)
                                ptil[2 * pi] = pt[:, 0:QC]
                                ptil[2 * pi + 1] = pt[:, QC:2 * QC]
                                if 4 * j + l < 8:
                                    pop_deferred()
                            for kt in range(2 * NPAIR, KT):
                                ps = psA.tile([128, 512], F32, tag="psA")
                                nc.tensor.matmul(ps[:, :],
                                                 ktil[ftl][hb:hb + 64, 128 * kt:128 * (kt + 1)],
                                                 qtil[ftl][hb:hb + 64, QC * j:QC * (j + 1)],
                                                 start=True, stop=True)
                                pt = ppool.tile([128, QC], BF16, tag="pts", bufs=8, name="pt2")
                                nc.scalar.activation(pt[:, :], ps[:, :], AF.Exp,
                                                     bias=kb[:, kt:kt + 1])
                                ptil[kt] = pt[:, :]
                                if 4 * j + l < 8:
                                    pop_deferred()
                            pv = psPV.tile([68, QC], F32, tag="pv")
                            for kt in range(KT):
                                nc.tensor.matmul(pv[:, :],
                                                 vaug[kt][:, 81 * l:81 * l + 68],
                                                 ptil[kt],
                                                 start=(kt == 0), stop=(kt == KT - 1))
                            # PE density padding: keeps the HAM activity monitor
                            # from re-throttling during the ACT-bound stretch
                            if 4 * j + l >= 8:
                                for _w in range(6):
                                    wps = psA.tile([128, 512], F32, tag="psA")
                                    nc.tensor.matmul(wps[:, :], ktil[0][0:64, 0:128],
                                                     qtil[0][0:64, 0:512],
                                                     start=True, stop=True)
                            nc.vector.tensor_copy(out=usb[4 * (j % 2) + l][:, :],
                                                  in_=pv[0:64, :])
                            nc.vector.tensor_tensor(out=dblk[j][64:68, :],
                                                    in0=dblk[j][64:68, :],
                                                    in1=pv[64:68, :], op=OP.add)
                            if l == 0 and pending is not None:
                                flush(*pending)
                                pending = None
                        # ---- sender-side softmax normalization for chunk j ----
                        dacc = a0pool.tile([128, QC], F32, tag="dacc", bufs=2, name="dacc")
                        nc.vector.tensor_tensor(out=dacc[64:68, :], in0=dblk[j][64:68, :],
                                                in1=qinf[64:68, QC * j:QC * (j + 1)],
                                                op=OP.add)
                        nc.vector.tensor_scalar(out=dacc[64:68, :], in0=dacc[64:68, :],
                                                scalar1=1e-30, scalar2=None, op0=OP.max)
                        rrj = a0pool.tile([128, QC], F32, tag="rrj", bufs=2, name="rrj")
                        nc.vector.reciprocal(out=rrj[64:68, :], in_=dacc[64:68, :])
                        rmk = a0pool.tile([128, QC], BF16, tag="rmk", bufs=2, name="rmk")
                        nc.vector.tensor_copy(out=rmk[64:68, :], in_=rrj[64:68, :])
                        pending = (j, rmk)
                    # cover the last chunk's reciprocal latency with PE filler,
                    # then flush it and fire the collective
                    for _w in range(40):
                        wps = psA.tile([128, 512], F32, tag="psA")
                        nc.tensor.matmul(wps[:, :], ktil[0][0:64, 0:128],
                                         qtil[0][0:64, 0:512], start=True, stop=True)
                    flush(*pending)
                nc.gpsimd.collective_compute(
                    "AllToAll", OP.bypass,
                    replica_groups=[[0, 1, 2, 3, 4, 5, 6, 7]],
                    ins=[shard.opt()], outs=[gath.opt()])

                # keep the PE HAM clock warm across the collective window
                for _w in range(NDUMMY):
                    wps = psA.tile([128, 512], F32, tag="psA")
                    nc.tensor.matmul(wps[:, :], ktil[0][0:64, 0:128],
                                     qtil[0][0:64, 0:512], start=True, stop=True)

            # ---------------- receiver: projection ----------------
            with tc.tile_pool(name="attn", bufs=1) as apool:
                u2 = [apool.tile([128, QC], BF16, tag=f"u2{p}", name=f"u2{p}")
                      for p in range(8)]
                for s in range(4):
                    for t in range(2):
                        p = 2 * s + t
                        ua = apool.tile([128, QC], BF16, tag="ua", bufs=4, name="ua")
                        ub = apool.tile([128, QC], BF16, tag="ub", bufs=4, name="ub")
                        nc.sync.dma_start(
                            out=ua[:, :],
                            in_=gath[s * BSn + 2 * 64 * QC * t:s * BSn + 2 * 64 * QC * (t + 1)])
                        nc.gpsimd.dma_start(
                            out=ub[:, :],
                            in_=gath[(4 + s) * BSn + 2 * 64 * QC * t:(4 + s) * BSn + 2 * 64 * QC * (t + 1)])
                        nc.vector.tensor_tensor(out=u2[p][:, :], in0=ua[:, :],
                                                in1=ub[:, :], op=OP.add)
                for mt in range(4):
                    outsb = apool.tile([128, D], F32, tag="outsb", bufs=2)
                    for ch in range(2):
                        ps = psBig.tile([128, 2 * QC], F32, tag="psBig")
                        for p in range(8):
                            nc.tensor.matmul(ps[:, 0:512],
                                             u2[p][:, 128 * mt:128 * (mt + 1)],
                                             wp2[p][:, 512 * ch:512 * (ch + 1)],
                                             start=(p == 0), stop=False)
                        nc.tensor.matmul(ps[:, 0:512], onesq[:, 128 * mt:128 * (mt + 1)],
                                         bprow[:, 512 * ch:512 * (ch + 1)],
                                         start=False, stop=False)
                        nc.tensor.matmul(ps[:, 0:512], iqrow[:, 128 * mt:128 * (mt + 1)],
                                         fixrow[:, 512 * ch:512 * (ch + 1)],
                                         start=False, stop=True)
                        nc.vector.tensor_copy(out=outsb[:, 512 * ch:512 * (ch + 1)],
                                              in_=ps[:, 0:512])
                    nc.sync.dma_start(out=out_d[128 * mt:128 * (mt + 1), :],
                                      in_=outsb[:, :])
    nc.compile()
    return nc


def _prep(x, vaild_num, W_qkv, b_qkv, W_proj, b_proj):
    v = np.asarray(vaild_num).astype(np.int64)
    vmax = int(max(1, v.max()))
    KT = (vmax + 127) // 128
    BT0 = min(int(v.min()) // 128, KT)
    wq = W_qkv[:, 0:D]
    wk = W_qkv[:, D:2 * D]
    wv = W_qkv[:, 2 * D:3 * D]
    wproj_bf = np.ascontiguousarray(W_proj.astype(BF))
    bq = b_qkv[0:D]
    bk = b_qkv[D:2 * D]
    bv = b_qkv[2 * D:3 * D]
    kiota = (np.arange(128, dtype=np.float32)[:, None]
             + 128.0 * np.arange(KT, dtype=np.float32)[None, :])
    bprow = np.ascontiguousarray(b_proj.reshape(1, D).astype(BF))
    in_maps = []
    for c in range(NCORES):
        b, r = c // 4, c % 4
        q0 = QC * r
        xTb = np.ascontiguousarray(x[b].T.astype(BF))
        sl = slice(256 * r, 256 * (r + 1))
        meanV = x[b].mean(axis=0).astype(np.float32) @ wv + bv
        fixrow_np = np.ascontiguousarray((meanV @ W_proj).reshape(1, D).astype(BF))
        # broadcast selector, flag-scaled per A2A half: half 0 live iff b==0
        esel4_np = np.zeros((4, 512), BF)
        for l in range(4):
            esel4_np[l, 64 * l:64 * (l + 1)] = 1.0 - b
            esel4_np[l, 256 + 64 * l:256 + 64 * (l + 1)] = float(b)
        qinf_np = np.broadcast_to(
            ((np.arange(N) >= v[b]) * 1e30).astype(np.float32)[None, :], (4, N)).copy()
        iqrow_np = np.ascontiguousarray(
            ((q0 + np.arange(QC)) >= v[b]).astype(BF).reshape(1, QC))
        m = {
            "xT": xTb,
            "wqmy": np.ascontiguousarray(wq[:, sl].astype(BF)),
            "wkmy": np.ascontiguousarray(wk[:, sl].astype(BF)),
            "wvmy": np.ascontiguousarray(wv[:, sl].astype(BF)),
            "bqmy": np.ascontiguousarray(
                (bq[sl] / 8.0).reshape(2, 128).T.astype(np.float32)),
            "bkmy": np.ascontiguousarray(
                bk[sl].reshape(2, 128).T.astype(np.float32)),
            "bvrowmy": np.ascontiguousarray(bv[sl].reshape(1, 256).astype(BF)),
            "wproj": wproj_bf,
            "bprow": bprow,
            "fixrow": fixrow_np,
            "iqrow": iqrow_np,
            "qinf": qinf_np,
            "esel4": esel4_np,
            "v128": np.full((128, 1), float(v[b]), np.float32),
            "kiota": kiota,
        }
        in_maps.append(m)
    return KT, BT0, in_maps


def _install_ntff_hook():
    """Provide antenv.axon_hooks backed by trn_boot's ctypes NTFF profiler."""
    import sys, types
    try:
        from antenv import axon_hooks  # noqa: F401
        return
    except ImportError:
        pass
    mod = types.ModuleType("antenv.axon_hooks")
    _h = [None]
    mod.set_axon_ntff_profile_hook = lambda h: _h.__setitem__(0, h)
    mod.get_axon_ntff_profile_hook = lambda: _h[0]
    sys.modules["antenv.axon_hooks"] = mod
    try:
        from trn_agent_boot.trn_boot import _ntff_profile_via_ctypes
        hook = _ntff_profile_via_ctypes("/opt/axon/libaxon_pjrt.so")
        mod.set_axon_ntff_profile_hook(hook)
    except Exception as e:  # profiling degrades, run still works
        print("ntff hook install failed:", e)


_CACHE = {}


def kernel(x, vaild_num, W_qkv, b_qkv, W_proj, b_proj, _trace=False):
    x = np.asarray(x, np.float32)
    KT, BT0, in_maps = _prep(np.asarray(x, np.float32), vaild_num,
                             np.asarray(W_qkv, np.float32), np.asarray(b_qkv, np.float32),
                             np.asarray(W_proj, np.float32), np.asarray(b_proj, np.float32))
    _install_ntff_hook()
    if (KT, BT0) not in _CACHE:
        _CACHE[(KT, BT0)] = build_nc(KT, BT0)
    nc = _CACHE[(KT, BT0)]
    res = run_bass_kernel_spmd(nc, in_maps, core_ids=list(range(NCORES)),
                               trace=_trace)
    out = np.empty((B, N, D), np.float32)
    for c in range(NCORES):
        b, j = c // 4, c % 4
        out[b, QC * j:QC * (j + 1), :] = res.results[c]["out"]
    kernel._last_exec_ns = res.exec_time_ns
    return out
